# revision 5
# baseline (speedup 1.0000x reference)
"""Trainium2 Bass kernel for NPChangeTransitionPrior (dense per-dim MLP + Jacobian logdet).

kernel(**inputs) takes FULL unsharded inputs (as in setup_inputs()) and returns
(residuals [B, length, D], log_abs_det_jacobian [B, length]).
Internally: batch-sharded data-parallel across 8 NeuronCores via one SPMD Bass program.

Per row n, per latent dim d (D=16, H=64):
  z0 = W0_d @ [emb, x_d] + b0_d ; a_k = leaky(z_k); g_k = gate(z_k)  (3 layers)
  res_d = Wo_d @ a2 + bo_d
  t0 = g0*W0_d[:,-1]; t_k = g_k * (W_k @ t_{k-1}); dJ_d = Wo_d @ t2
  logdet = sum_d log|dJ_d|

Layout: activations feature-major [128 partitions = dim-pair x 64 hidden, n cols].
Everything fp32 (Jacobian logdet is log-amplified near cancellation; reduced-precision
matmuls flip leaky-relu gates and fail absmax vs the fp32 reference).
"""

import sys
import numpy as np

for _p in ("/opt/trn_rl_repo", "/root/.axon_site/_ro/trn_rl_repo"):
    if _p not in sys.path:
        sys.path.insert(0, _p)

import concourse.bacc as bacc
import concourse.tile as tile
from concourse import mybir
from concourse import bass2jax

F32 = mybir.dt.float32
F32R = mybir.dt.float32r
AF = mybir.ActivationFunctionType
ALU = mybir.AluOpType

B, T, D, E, H = 32, 1002, 16, 32, 64
LAGS = 2
LEN = T - LAGS
NCORES = 8
BC = B // NCORES          # 4 batch rows per core
N = BC * LEN              # 4000 rows per core
C = 512                   # columns per chunk
NCH = (N + C - 1) // C    # 8 chunks
NP = NCH * C              # 4096 padded rows
NPAIR = D // 2            # 8 dim-pairs
SLOPE = 0.2
K0 = E + D + 1            # 49 rows of the layer-0 rhs (emb, x, ones)

_CACHE = {}


def _emit_chunks(nc, tc, tensors, reps):
    (rhs0, lhsT0, w1b, w2b, who, ones16, bv1, bv2, w0c, bo16, res16, ldet) = tensors
    with (
        tc.tile_pool(name="wpool", bufs=1) as wp,
        tc.tile_pool(name="io", bufs=3) as io,
        tc.tile_pool(name="spool", bufs=4) as sp,
        tc.tile_pool(name="gpool", bufs=4) as gp,
        tc.tile_pool(name="epool", bufs=3) as ep,
        tc.tile_pool(name="opool", bufs=3) as op,
        tc.tile_pool(name="psz", bufs=5, space="PSUM") as psz,
        tc.tile_pool(name="psacc", bufs=1, space="PSUM") as psacc,
    ):
        lhsT0_t = wp.tile([K0, NPAIR * 128], F32)
        w1b_t = wp.tile([128, NPAIR * 128], F32)
        w2b_t = wp.tile([128, NPAIR * 128], F32)
        who_t = wp.tile([128, NPAIR * 16], F32)
        ones_t = wp.tile([16, 1], F32)
        bv1_t = wp.tile([128, NPAIR], F32)
        bv2_t = wp.tile([128, NPAIR], F32)
        w0c_t = wp.tile([128, NPAIR], F32)
        bo_t = wp.tile([16, 1], F32)
        for dst, src in ((lhsT0_t, lhsT0), (w1b_t, w1b), (w2b_t, w2b), (who_t, who),
                         (ones_t, ones16), (bv1_t, bv1), (bv2_t, bv2), (w0c_t, w0c),
                         (bo_t, bo16)):
            nc.sync.dma_start(out=dst[:], in_=src[:])

        for rep in range(reps):
            for ch in range(NCH):
                r0 = io.tile([K0, C], F32, tag="r0")
                nc.sync.dma_start(out=r0[:], in_=rhs0[:, ch * C:(ch + 1) * C])
                res_ps = psacc.tile([16, C], F32, tag="racc", bufs=1)
                dJ_ps = psacc.tile([16, C], F32, tag="jacc", bufs=1)

                for p in range(NPAIR):
                    w1p = w1b_t[:, p * 128:(p + 1) * 128]
                    w2p = w2b_t[:, p * 128:(p + 1) * 128]
                    whp = who_t[:, p * 16:(p + 1) * 16]

                    z0 = psz.tile([128, C], F32, tag="z", bufs=5)
                    nc.tensor.matmul(z0[:], lhsT0_t[:, p * 128:(p + 1) * 128], r0[:],
                                     start=True, stop=True)
                    a0 = sp.tile([128, C], F32, tag="a")
                    t0 = sp.tile([128, C], F32, tag="t")
                    g0 = gp.tile([128, C], F32, tag="g")
                    nc.scalar.activation(a0[:], z0[:], AF.Prelu, bias=0.0, scale=1.0,
                                         alpha=SLOPE)
                    nc.vector.tensor_scalar(g0[:], a0[:], 0.0, SLOPE, ALU.is_ge, ALU.max)
                    nc.vector.tensor_scalar(t0[:], g0[:], w0c_t[:, p:p + 1], None, ALU.mult)

                    z1 = psz.tile([128, C], F32, tag="z", bufs=5)
                    u1 = psz.tile([128, C], F32, tag="z", bufs=5)
                    nc.tensor.matmul(z1[:], w1p, a0[:], start=True, stop=True)
                    nc.tensor.matmul(u1[:], w1p, t0[:], start=True, stop=True)
                    a1 = sp.tile([128, C], F32, tag="a")
                    t1 = sp.tile([128, C], F32, tag="t")
                    g1 = gp.tile([128, C], F32, tag="g")
                    nc.scalar.activation(a1[:], z1[:], AF.Prelu, bias=bv1_t[:, p:p + 1],
                                         scale=1.0, alpha=SLOPE)
                    nc.vector.tensor_scalar(g1[:], a1[:], 0.0, SLOPE, ALU.is_ge, ALU.max)
                    nc.vector.tensor_tensor(t1[:], g1[:], u1[:], ALU.mult)

                    z2 = psz.tile([128, C], F32, tag="z", bufs=5)
                    u2 = psz.tile([128, C], F32, tag="z", bufs=5)
                    nc.tensor.matmul(z2[:], w2p, a1[:], start=True, stop=True)
                    nc.tensor.matmul(u2[:], w2p, t1[:], start=True, stop=True)
                    a2 = sp.tile([128, C], F32, tag="a")
                    t2 = sp.tile([128, C], F32, tag="t")
                    g2 = gp.tile([128, C], F32, tag="g")
                    nc.scalar.activation(a2[:], z2[:], AF.Prelu, bias=bv2_t[:, p:p + 1],
                                         scale=1.0, alpha=SLOPE)
                    nc.vector.tensor_scalar(g2[:], a2[:], 0.0, SLOPE, ALU.is_ge, ALU.max)
                    nc.vector.tensor_tensor(t2[:], g2[:], u2[:], ALU.mult)

                    nc.tensor.matmul(res_ps[:], whp, a2[:],
                                     start=(p == 0), stop=(p == NPAIR - 1))
                    nc.tensor.matmul(dJ_ps[:], whp, t2[:],
                                     start=(p == 0), stop=(p == NPAIR - 1))

                res_sb = op.tile([16, C], F32, tag="res_sb")
                nc.scalar.activation(res_sb[:], res_ps[:], AF.Identity,
                                     bias=bo_t[:, 0:1], scale=1.0)
                nc.sync.dma_start(out=res16[:, ch * C:(ch + 1) * C], in_=res_sb[:])

                sq = ep.tile([16, C], F32, tag="sq")
                nc.scalar.activation(sq[:], dJ_ps[:], AF.Square, bias=0.0, scale=1.0)
                ln = ep.tile([16, C], F32, tag="ln")
                nc.scalar.activation(ln[:], sq[:], AF.Ln, bias=0.0, scale=1.0)
                ld_ps = psz.tile([1, C], F32, tag="ldps", bufs=1)
                nc.tensor.matmul(ld_ps[:], ones_t[:], ln[:], start=True, stop=True)
                ld_sb = op.tile([1, C], F32, tag="ld_sb")
                nc.scalar.activation(ld_sb[:], ld_ps[:], AF.Copy, bias=0.0, scale=0.5)
                nc.sync.dma_start(out=ldet[:, ch * C:(ch + 1) * C], in_=ld_sb[:])


def _build_program(reps=1):
    nc = bacc.Bacc()
    rhs0 = nc.dram_tensor("rhs0", [K0, NP], F32, kind="ExternalInput")
    lhsT0 = nc.dram_tensor("lhsT0", [K0, NPAIR * 128], F32, kind="ExternalInput")
    w1b = nc.dram_tensor("w1b", [128, NPAIR * 128], F32, kind="ExternalInput")
    w2b = nc.dram_tensor("w2b", [128, NPAIR * 128], F32, kind="ExternalInput")
    who = nc.dram_tensor("who", [128, NPAIR * 16], F32, kind="ExternalInput")
    ones16 = nc.dram_tensor("ones16", [16, 1], F32, kind="ExternalInput")
    bv1 = nc.dram_tensor("bv1", [128, NPAIR], F32, kind="ExternalInput")
    bv2 = nc.dram_tensor("bv2", [128, NPAIR], F32, kind="ExternalInput")
    w0c = nc.dram_tensor("w0c", [128, NPAIR], F32, kind="ExternalInput")
    bo16 = nc.dram_tensor("bo16", [16, 1], F32, kind="ExternalInput")
    res16 = nc.dram_tensor("res16", [16, NP], F32, kind="ExternalOutput")
    ldet = nc.dram_tensor("ldet", [1, NP], F32, kind="ExternalOutput")
    tensors = (rhs0, lhsT0, w1b, w2b, who, ones16, bv1, bv2, w0c, bo16, res16, ldet)
    with tile.TileContext(nc) as tc:
        _emit_chunks(nc, tc, tensors, reps)
    nc.finalize()
    return nc


def _prep_weights(W0, b0, W1, b1, W2, b2, Wo, bo):
    lhsT0 = np.zeros((K0, NPAIR * 128), np.float32)
    w1b = np.zeros((128, NPAIR * 128), np.float32)
    w2b = np.zeros((128, NPAIR * 128), np.float32)
    who = np.zeros((128, NPAIR * 16), np.float32)
    bv1 = np.zeros((128, NPAIR), np.float32)
    bv2 = np.zeros((128, NPAIR), np.float32)
    w0c = np.zeros((128, NPAIR), np.float32)
    for p in range(NPAIR):
        d0, d1 = 2 * p, 2 * p + 1
        c = p * 128
        lhsT0[0:E, c:c + 64] = W0[d0, :, :E].T
        lhsT0[0:E, c + 64:c + 128] = W0[d1, :, :E].T
        lhsT0[E + d0, c:c + 64] = W0[d0, :, E]
        lhsT0[E + d1, c + 64:c + 128] = W0[d1, :, E]
        lhsT0[K0 - 1, c:c + 64] = b0[d0]
        lhsT0[K0 - 1, c + 64:c + 128] = b0[d1]
        w1b[0:64, c:c + 64] = W1[d0].T
        w1b[64:128, c + 64:c + 128] = W1[d1].T
        w2b[0:64, c:c + 64] = W2[d0].T
        w2b[64:128, c + 64:c + 128] = W2[d1].T
        who[0:64, p * 16 + d0] = Wo[d0, 0, :]
        who[64:128, p * 16 + d1] = Wo[d1, 0, :]
        bv1[0:64, p] = b1[d0]
        bv1[64:128, p] = b1[d1]
        bv2[0:64, p] = b2[d0]
        bv2[64:128, p] = b2[d1]
        w0c[0:64, p] = W0[d0, :, E]
        w0c[64:128, p] = W0[d1, :, E]
    return {
        "lhsT0": lhsT0, "w1b": w1b, "w2b": w2b, "who": who,
        "ones16": np.ones((16, 1), np.float32),
        "bv1": bv1, "bv2": bv2, "w0c": w0c,
        "bo16": bo[:, 0:1].astype(np.float32).copy(),
    }


class _Runner:
    """Persistent shard_map executable over 8 cores (avoids per-call jit retrace)."""

    def __init__(self, nc):
        import jax
        from jax.sharding import Mesh, PartitionSpec
        from jax.experimental.shard_map import shard_map

        self.jax = jax
        bass2jax.install_neuronx_cc_hook()
        pname = nc.partition_id_tensor.name if nc.partition_id_tensor else None
        self.in_names, self.out_names, self.out_avals = [], [], []
        for alloc in nc.m.functions[0].allocations:
            if not isinstance(alloc, mybir.MemoryLocationSet):
                continue
            name = alloc.memorylocations[0].name
            if alloc.kind == "ExternalInput":
                if name != pname:
                    self.in_names.append(name)
            elif alloc.kind == "ExternalOutput":
                self.out_names.append(name)
                self.out_avals.append(
                    jax.core.ShapedArray(tuple(alloc.tensor_shape),
                                         mybir.dt.np(alloc.dtype)))
        n_params = len(self.in_names)
        n_outs = len(self.out_avals)
        all_names = list(self.in_names) + self.out_names + ([pname] if pname else [])
        out_avals = tuple(self.out_avals)
        out_names = tuple(self.out_names)

        def _body(*args):
            ops = list(args)
            if pname is not None:
                ops.append(bass2jax.partition_id_tensor())
            return tuple(bass2jax._bass_exec_p.bind(
                *ops, out_avals=out_avals, in_names=tuple(all_names),
                out_names=out_names, lowering_input_output_aliases=(),
                sim_require_finite=True, sim_require_nnan=True, nc=nc))

        devices = jax.devices()[:NCORES]
        mesh = Mesh(np.asarray(devices), ("core",))
        in_specs = (PartitionSpec("core"),) * (n_params + n_outs)
        out_specs = (PartitionSpec("core"),) * n_outs
        self.fn = jax.jit(
            shard_map(_body, mesh=mesh, in_specs=in_specs, out_specs=out_specs,
                      check_rep=False),
            keep_unused=True)
        self.zero_outs = [np.zeros((NCORES * a.shape[0], *a.shape[1:]), a.dtype)
                          for a in self.out_avals]

    def __call__(self, per_core_maps):
        cat = [np.concatenate([np.asarray(m[name]) for m in per_core_maps], axis=0)
               for name in self.in_names]
        outs = self.fn(*cat, *self.zero_outs)
        self.jax.block_until_ready(outs)
        return {name: np.asarray(outs[i]).reshape(NCORES, *self.out_avals[i].shape)
                for i, name in enumerate(self.out_names)}


def _get_runner(reps=1):
    key = ("runner", reps)
    if key not in _CACHE:
        _CACHE[key] = _Runner(_build_program(reps))
    return _CACHE[key]


def _make_in_maps(x, embeddings, wmap):
    in_maps = []
    for k in range(NCORES):
        xs = x[k * BC:(k + 1) * BC, LAGS:, :].reshape(N, D)
        es = embeddings[k * BC:(k + 1) * BC, LAGS:, :].reshape(N, E)
        rhs0 = np.zeros((K0, NP), np.float32)
        rhs0[0:E, :N] = es.T
        rhs0[E:E + D, :N] = xs.T
        rhs0[E + D, :] = 1.0
        m = dict(wmap)
        m["rhs0"] = rhs0
        in_maps.append(m)
    return in_maps


def kernel(x, embeddings, W0, b0, W1, b1, W2, b2, Wo, bo):
    x = np.asarray(x, np.float32)
    embeddings = np.asarray(embeddings, np.float32)
    wmap = _prep_weights(*[np.asarray(a, np.float32)
                           for a in (W0, b0, W1, b1, W2, b2, Wo, bo)])
    runner = _get_runner()
    outs = runner(_make_in_maps(x, embeddings, wmap))

    residuals = np.empty((B, LEN, D), np.float32)
    logdet = np.empty((B, LEN), np.float32)
    for k in range(NCORES):
        residuals[k * BC:(k + 1) * BC] = outs["res16"][k][:, :N].T.reshape(BC, LEN, D)
        logdet[k * BC:(k + 1) * BC] = outs["ldet"][k][0, :N].reshape(BC, LEN)
    return residuals, logdet


# revision 8
# speedup vs baseline: 56.3323x; 56.3323x over previous
"""Trainium2 Bass kernel for NPChangeTransitionPrior (dense per-dim MLP + Jacobian logdet).

kernel(**inputs) takes FULL unsharded inputs (as in setup_inputs()) and returns
(residuals [B, length, D], log_abs_det_jacobian [B, length]).
Internally: batch-sharded data-parallel across 8 NeuronCores via one SPMD Bass program.

Per row n, per latent dim d (D=16, H=64):
  z0 = W0_d @ [emb, x_d] + b0_d ; a_k = leaky(z_k); g_k = gate(z_k)  (3 layers)
  res_d = Wo_d @ a2 + bo_d
  t0 = g0*W0_d[:,-1]; t_k = g_k * (W_k @ t_{k-1}); dJ_d = Wo_d @ t2
  logdet = sum_d log|dJ_d|

Layout: activations feature-major [128 partitions = dim-pair x 64 hidden, n cols].
Everything fp32 (Jacobian logdet is log-amplified near cancellation; reduced-precision
matmuls flip leaky-relu gates and fail absmax vs the fp32 reference).
"""

import sys
import numpy as np

for _p in ("/opt/trn_rl_repo", "/root/.axon_site/_ro/trn_rl_repo"):
    if _p not in sys.path:
        sys.path.insert(0, _p)

import concourse.bacc as bacc
import concourse.tile as tile
from concourse import mybir
from concourse import bass2jax

F32 = mybir.dt.float32
F32R = mybir.dt.float32r
AF = mybir.ActivationFunctionType
ALU = mybir.AluOpType

B, T, D, E, H = 32, 1002, 16, 32, 64
LAGS = 2
LEN = T - LAGS
NCORES = 8
BC = B // NCORES          # 4 batch rows per core
N = BC * LEN              # 4000 rows per core
C = 512                   # columns per chunk
NCH = (N + C - 1) // C    # 8 chunks
NP = NCH * C              # 4096 padded rows
NPAIR = D // 2            # 8 dim-pairs
SLOPE = 0.2
ZBUFS = 4
HBUFS = 2
K0 = E + D + 1            # 49 rows of the layer-0 rhs (emb, x, ones)

_CACHE = {}


def _emit_chunks(nc, tc, tensors, reps):
    """Quad (2x2 tile_position) emission: per group of 4 dims, each layer-chain is
    4 concurrent 64x64 fp32 matmuls occupying distinct PE array quadrants.
    Slab A holds dims (4g, 4g+1); slab B holds (4g+2, 4g+3); the B-slab order
    swaps after L1 (quadrant geometry) and swaps back after L2 -- the host-side
    weight/bias packing accounts for it."""
    (rhs0, lhsT0q, w1q, w2q, whoq, ones64, bv1q, bv2q, w0cq, boq, lnmask,
     res16, ldet) = tensors
    NG = D // 4  # 4 groups of 4 dims
    with (
        tc.tile_pool(name="wpool", bufs=1) as wp,
        tc.tile_pool(name="io", bufs=3) as io,
        tc.tile_pool(name="spool", bufs=4) as sp,
        tc.tile_pool(name="gpool", bufs=4) as gp,
        tc.tile_pool(name="epool", bufs=3) as ep,
        tc.tile_pool(name="opool", bufs=3) as op,
        tc.tile_pool(name="psz", bufs=ZBUFS, space="PSUM") as psz,
        tc.tile_pool(name="psh", bufs=HBUFS, space="PSUM") as psh,
    ):
        lhsT0q_t = wp.tile([128, NG * 256], F32)
        w1q_t = wp.tile([128, NG * 256], F32)
        w2q_t = wp.tile([128, NG * 256], F32)
        whoq_t = wp.tile([128, NG * 64], F32)
        ones_t = wp.tile([128, 1], F32)
        bv1q_t = wp.tile([128, NG * 2], F32)
        bv2q_t = wp.tile([128, NG * 2], F32)
        w0cq_t = wp.tile([128, NG * 2], F32)
        boq_t = wp.tile([128, NG], F32)
        lnm_t = wp.tile([128, 1], F32)
        for dst, src_ in ((lhsT0q_t, lhsT0q), (w1q_t, w1q), (w2q_t, w2q),
                          (whoq_t, whoq), (ones_t, ones64), (bv1q_t, bv1q),
                          (bv2q_t, bv2q), (w0cq_t, w0cq), (boq_t, boq),
                          (lnm_t, lnmask)):
            nc.sync.dma_start(out=dst[:], in_=src_[:])

        def quad_layer(wt, gbase, inA, inB, outA, outB, k0=None):
            """4 concurrent 64x64 mms. wt blocks at cols gbase+{0,64,128,192}.
            k0: contraction rows per tile (None -> 64)."""
            k = k0 if k0 is not None else 64
            nc.tensor.matmul(outA[0:64], wt[0:k, gbase + 0:gbase + 64],
                             inA[0:k], start=True, stop=True)
            nc.tensor.matmul(outA[64:128], wt[64:64 + k, gbase + 64:gbase + 128],
                             inA[64:64 + k], start=True, stop=True)
            nc.tensor.matmul(outB[0:64], wt[64:64 + k, gbase + 128:gbase + 192],
                             inB[64:64 + k], start=True, stop=True)
            nc.tensor.matmul(outB[64:128], wt[0:k, gbase + 192:gbase + 256],
                             inB[0:k], start=True, stop=True)

        def gates_and_t(zA, zB, uA, uB, bcolA, bcolB, aA, aB, tA, tB, gA, gB,
                        w0colA=None, w0colB=None, g_engines=("v", "v")):
            """Per-slab activation/gate/t ops. u=None at L0 (t = g * w0col)."""
            for (z, u, bcol, a, t, g, w0col, geng) in (
                (zA, uA, bcolA, aA, tA, gA, w0colA, g_engines[0]),
                (zB, uB, bcolB, aB, tB, gB, w0colB, g_engines[1]),
            ):
                nc.scalar.activation(a[:], z[:], AF.Prelu, bias=bcol, scale=1.0,
                                     alpha=SLOPE)
                eng = nc.vector if geng == "v" else nc.gpsimd
                eng.tensor_scalar(g[:], a[:], 0.0, SLOPE, ALU.is_ge, ALU.max)
                if u is None:
                    nc.vector.tensor_scalar(t[:], g[:], w0col, None, ALU.mult)
                else:
                    nc.vector.tensor_tensor(t[:], g[:], u[:], ALU.mult)

        for rep in range(reps):
            for ch in range(NCH):
                r0 = io.tile([128, C], F32, tag="r0")
                nc.sync.dma_start(out=r0[0:K0, :], in_=rhs0[:, ch * C:(ch + 1) * C])
                nc.sync.dma_start(out=r0[64:64 + K0, :], in_=rhs0[:, ch * C:(ch + 1) * C])
                ld_ps = psz.tile([1, C], F32, tag="ldps", bufs=1)

                for g in range(NG):
                    gb = g * 256
                    z0A = psz.tile([128, C], F32, tag="z", bufs=ZBUFS)
                    z0B = psz.tile([128, C], F32, tag="z", bufs=ZBUFS)
                    quad_layer(lhsT0q_t, gb, r0, r0, z0A, z0B, k0=K0)
                    a0A = sp.tile([128, C], F32, tag="a")
                    a0B = sp.tile([128, C], F32, tag="a")
                    t0A = sp.tile([128, C], F32, tag="t")
                    t0B = sp.tile([128, C], F32, tag="t")
                    g0A = gp.tile([128, C], F32, tag="g")
                    g0B = gp.tile([128, C], F32, tag="g")
                    gates_and_t(z0A, z0B, None, None, 0.0, 0.0, a0A, a0B,
                                t0A, t0B, g0A, g0B,
                                w0colA=w0cq_t[:, 2 * g:2 * g + 1],
                                w0colB=w0cq_t[:, 2 * g + 1:2 * g + 2],
                                g_engines=("p", "p"))

                    z1A = psz.tile([128, C], F32, tag="z", bufs=ZBUFS)
                    z1B = psz.tile([128, C], F32, tag="z", bufs=ZBUFS)
                    quad_layer(w1q_t, gb, a0A, a0B, z1A, z1B)
                    u1A = psz.tile([128, C], F32, tag="z", bufs=ZBUFS)
                    u1B = psz.tile([128, C], F32, tag="z", bufs=ZBUFS)
                    quad_layer(w1q_t, gb, t0A, t0B, u1A, u1B)
                    a1A = sp.tile([128, C], F32, tag="a")
                    a1B = sp.tile([128, C], F32, tag="a")
                    t1A = sp.tile([128, C], F32, tag="t")
                    t1B = sp.tile([128, C], F32, tag="t")
                    g1A = gp.tile([128, C], F32, tag="g")
                    g1B = gp.tile([128, C], F32, tag="g")
                    gates_and_t(z1A, z1B, u1A, u1B,
                                bv1q_t[:, 2 * g:2 * g + 1], bv1q_t[:, 2 * g + 1:2 * g + 2],
                                a1A, a1B, t1A, t1B, g1A, g1B, g_engines=("p", "v"))

                    z2A = psz.tile([128, C], F32, tag="z", bufs=ZBUFS)
                    z2B = psz.tile([128, C], F32, tag="z", bufs=ZBUFS)
                    quad_layer(w2q_t, gb, a1A, a1B, z2A, z2B)
                    u2A = psz.tile([128, C], F32, tag="z", bufs=ZBUFS)
                    u2B = psz.tile([128, C], F32, tag="z", bufs=ZBUFS)
                    quad_layer(w2q_t, gb, t1A, t1B, u2A, u2B)
                    a2A = sp.tile([128, C], F32, tag="a")
                    a2B = sp.tile([128, C], F32, tag="a")
                    t2A = sp.tile([128, C], F32, tag="t")
                    t2B = sp.tile([128, C], F32, tag="t")
                    g2A = gp.tile([128, C], F32, tag="g")
                    g2B = gp.tile([128, C], F32, tag="g")
                    gates_and_t(z2A, z2B, u2A, u2B,
                                bv2q_t[:, 2 * g:2 * g + 1], bv2q_t[:, 2 * g + 1:2 * g + 2],
                                a2A, a2B, t2A, t2B, g2A, g2B, g_engines=("p", "v"))

                    # heads: one bank per group; rows 0:2 res(dA,dB), 32:34 res(dC,dD),
                    # 64:66 dJ(dA,dB), 96:98 dJ(dC,dD); M=32 writes zeros elsewhere
                    H = psh.tile([128, C], F32, tag="H", bufs=HBUFS)
                    wA = whoq_t[:, g * 64:g * 64 + 32]
                    wB = whoq_t[:, g * 64 + 32:g * 64 + 64]
                    nc.tensor.matmul(H[0:32], wA, a2A[:], start=True, stop=True,
                                     tile_position=(0, 0))
                    nc.tensor.matmul(H[32:64], wB, a2B[:], start=True, stop=True,
                                     tile_position=(0, 32))
                    nc.tensor.matmul(H[64:96], wA, t2A[:], start=True, stop=True,
                                     tile_position=(0, 64))
                    nc.tensor.matmul(H[96:128], wB, t2B[:], start=True, stop=True,
                                     tile_position=(0, 96))

                    # epilogue for this group
                    res_sb = op.tile([64, C], F32, tag="res_sb")
                    nc.scalar.activation(res_sb[:], H[0:64], AF.Identity,
                                         bias=boq_t[0:64, g:g + 1], scale=1.0)
                    nc.sync.dma_start(out=res16[4 * g:4 * g + 2, ch * C:(ch + 1) * C],
                                      in_=res_sb[0:2, :])
                    nc.sync.dma_start(out=res16[4 * g + 2:4 * g + 4, ch * C:(ch + 1) * C],
                                      in_=res_sb[32:34, :])

                    sq = ep.tile([128, C], F32, tag="sq")
                    nc.scalar.activation(sq[64:128], H[64:128], AF.Square,
                                         bias=0.0, scale=1.0)
                    ln = ep.tile([128, C], F32, tag="ln")
                    nc.scalar.activation(ln[64:128], sq[64:128], AF.Ln,
                                         bias=lnm_t[64:128, 0:1], scale=1.0)
                    nc.tensor.matmul(ld_ps[:], ones_t[64:128, 0:1], ln[64:128],
                                     start=(g == 0), stop=(g == NG - 1))

                ld_sb = op.tile([1, C], F32, tag="ld_sb")
                nc.scalar.activation(ld_sb[:], ld_ps[:], AF.Copy, bias=0.0, scale=0.5)
                nc.sync.dma_start(out=ldet[:, ch * C:(ch + 1) * C], in_=ld_sb[:])


def _build_program(reps=1):
    nc = bacc.Bacc()
    NG = D // 4
    rhs0 = nc.dram_tensor("rhs0", [K0, NP], F32, kind="ExternalInput")
    lhsT0q = nc.dram_tensor("lhsT0q", [128, NG * 256], F32, kind="ExternalInput")
    w1q = nc.dram_tensor("w1q", [128, NG * 256], F32, kind="ExternalInput")
    w2q = nc.dram_tensor("w2q", [128, NG * 256], F32, kind="ExternalInput")
    whoq = nc.dram_tensor("whoq", [128, NG * 64], F32, kind="ExternalInput")
    ones64 = nc.dram_tensor("ones64", [128, 1], F32, kind="ExternalInput")
    bv1q = nc.dram_tensor("bv1q", [128, NG * 2], F32, kind="ExternalInput")
    bv2q = nc.dram_tensor("bv2q", [128, NG * 2], F32, kind="ExternalInput")
    w0cq = nc.dram_tensor("w0cq", [128, NG * 2], F32, kind="ExternalInput")
    boq = nc.dram_tensor("boq", [128, NG], F32, kind="ExternalInput")
    lnmask = nc.dram_tensor("lnmask", [128, 1], F32, kind="ExternalInput")
    res16 = nc.dram_tensor("res16", [16, NP], F32, kind="ExternalOutput")
    ldet = nc.dram_tensor("ldet", [1, NP], F32, kind="ExternalOutput")
    tensors = (rhs0, lhsT0q, w1q, w2q, whoq, ones64, bv1q, bv2q, w0cq, boq,
               lnmask, res16, ldet)
    with tile.TileContext(nc) as tc:
        _emit_chunks(nc, tc, tensors, reps)
    nc.finalize()
    return nc


def _prep_weights(W0, b0, W1, b1, W2, b2, Wo, bo):
    NG = D // 4
    lhsT0q = np.zeros((128, NG * 256), np.float32)
    w1q = np.zeros((128, NG * 256), np.float32)
    w2q = np.zeros((128, NG * 256), np.float32)
    whoq = np.zeros((128, NG * 64), np.float32)
    bv1q = np.zeros((128, NG * 2), np.float32)
    bv2q = np.zeros((128, NG * 2), np.float32)
    w0cq = np.zeros((128, NG * 2), np.float32)
    boq = np.zeros((128, NG), np.float32)

    def l0block(d):
        blk = np.zeros((K0, 64), np.float32)
        blk[0:E] = W0[d, :, :E].T
        blk[E + d] = W0[d, :, E]
        blk[K0 - 1] = b0[d]
        return blk

    for g in range(NG):
        dA, dB, dC, dD = 4 * g, 4 * g + 1, 4 * g + 2, 4 * g + 3
        gb = g * 256
        # L0: quad positions (0,0)=dA, (64,64)=dB, (64,0)=dC, (0,64)=dD
        lhsT0q[0:K0, gb + 0:gb + 64] = l0block(dA)
        lhsT0q[64:64 + K0, gb + 64:gb + 128] = l0block(dB)
        lhsT0q[64:64 + K0, gb + 128:gb + 192] = l0block(dC)
        lhsT0q[0:K0, gb + 192:gb + 256] = l0block(dD)
        # L1: inB natural [dC;dD] -> (64,0) takes inB-high=dD, (0,64) takes dC
        w1q[0:64, gb + 0:gb + 64] = W1[dA].T
        w1q[64:128, gb + 64:gb + 128] = W1[dB].T
        w1q[64:128, gb + 128:gb + 192] = W1[dD].T
        w1q[0:64, gb + 192:gb + 256] = W1[dC].T
        # L2: inB swapped [dD;dC] -> (64,0) takes inB-high=dC, (0,64) takes dD
        w2q[0:64, gb + 0:gb + 64] = W2[dA].T
        w2q[64:128, gb + 64:gb + 128] = W2[dB].T
        w2q[64:128, gb + 128:gb + 192] = W2[dC].T
        w2q[0:64, gb + 192:gb + 256] = W2[dD].T
        # biases per slab-column: A natural; L1-out B swapped, L2-out B natural
        bv1q[0:64, 2 * g] = b1[dA]
        bv1q[64:128, 2 * g] = b1[dB]
        bv1q[0:64, 2 * g + 1] = b1[dD]
        bv1q[64:128, 2 * g + 1] = b1[dC]
        bv2q[0:64, 2 * g] = b2[dA]
        bv2q[64:128, 2 * g] = b2[dB]
        bv2q[0:64, 2 * g + 1] = b2[dC]
        bv2q[64:128, 2 * g + 1] = b2[dD]
        # t0 = g0 * w0col (L0 outputs are natural order in both slabs)
        w0cq[0:64, 2 * g] = W0[dA, :, E]
        w0cq[64:128, 2 * g] = W0[dB, :, E]
        w0cq[0:64, 2 * g + 1] = W0[dC, :, E]
        w0cq[64:128, 2 * g + 1] = W0[dD, :, E]
        # heads: A-block cols (0,1) = Wo[dA] low / Wo[dB] high; B-block = dC/dD
        whoq[0:64, g * 64 + 0] = Wo[dA, 0, :]
        whoq[64:128, g * 64 + 1] = Wo[dB, 0, :]
        whoq[0:64, g * 64 + 32] = Wo[dC, 0, :]
        whoq[64:128, g * 64 + 33] = Wo[dD, 0, :]
        boq[0, g] = bo[dA, 0]
        boq[1, g] = bo[dB, 0]
        boq[32, g] = bo[dC, 0]
        boq[33, g] = bo[dD, 0]

    lnmask = np.ones((128, 1), np.float32)
    lnmask[[64, 65, 96, 97], 0] = 0.0
    return {
        "lhsT0q": lhsT0q, "w1q": w1q, "w2q": w2q, "whoq": whoq,
        "ones64": np.ones((128, 1), np.float32),
        "bv1q": bv1q, "bv2q": bv2q, "w0cq": w0cq, "boq": boq, "lnmask": lnmask,
    }


class _Runner:
    """Persistent shard_map executable over 8 cores (avoids per-call jit retrace)."""

    def __init__(self, nc):
        import jax
        from jax.sharding import Mesh, PartitionSpec
        from jax.experimental.shard_map import shard_map

        self.jax = jax
        bass2jax.install_neuronx_cc_hook()
        pname = nc.partition_id_tensor.name if nc.partition_id_tensor else None
        self.in_names, self.out_names, self.out_avals = [], [], []
        for alloc in nc.m.functions[0].allocations:
            if not isinstance(alloc, mybir.MemoryLocationSet):
                continue
            name = alloc.memorylocations[0].name
            if alloc.kind == "ExternalInput":
                if name != pname:
                    self.in_names.append(name)
            elif alloc.kind == "ExternalOutput":
                self.out_names.append(name)
                self.out_avals.append(
                    jax.core.ShapedArray(tuple(alloc.tensor_shape),
                                         mybir.dt.np(alloc.dtype)))
        n_params = len(self.in_names)
        n_outs = len(self.out_avals)
        all_names = list(self.in_names) + self.out_names + ([pname] if pname else [])
        out_avals = tuple(self.out_avals)
        out_names = tuple(self.out_names)

        def _body(*args):
            ops = list(args)
            if pname is not None:
                ops.append(bass2jax.partition_id_tensor())
            return tuple(bass2jax._bass_exec_p.bind(
                *ops, out_avals=out_avals, in_names=tuple(all_names),
                out_names=out_names, lowering_input_output_aliases=(),
                sim_require_finite=True, sim_require_nnan=True, nc=nc))

        devices = jax.devices()[:NCORES]
        mesh = Mesh(np.asarray(devices), ("core",))
        in_specs = (PartitionSpec("core"),) * (n_params + n_outs)
        out_specs = (PartitionSpec("core"),) * n_outs
        self.fn = jax.jit(
            shard_map(_body, mesh=mesh, in_specs=in_specs, out_specs=out_specs,
                      check_rep=False),
            keep_unused=True)
        self.zero_outs = [np.zeros((NCORES * a.shape[0], *a.shape[1:]), a.dtype)
                          for a in self.out_avals]

    def __call__(self, per_core_maps):
        cat = [np.concatenate([np.asarray(m[name]) for m in per_core_maps], axis=0)
               for name in self.in_names]
        outs = self.fn(*cat, *self.zero_outs)
        self.jax.block_until_ready(outs)
        return {name: np.asarray(outs[i]).reshape(NCORES, *self.out_avals[i].shape)
                for i, name in enumerate(self.out_names)}


def _get_runner(reps=1):
    key = ("runner", reps)
    if key not in _CACHE:
        _CACHE[key] = _Runner(_build_program(reps))
    return _CACHE[key]


def _make_in_maps(x, embeddings, wmap):
    in_maps = []
    for k in range(NCORES):
        xs = x[k * BC:(k + 1) * BC, LAGS:, :].reshape(N, D)
        es = embeddings[k * BC:(k + 1) * BC, LAGS:, :].reshape(N, E)
        rhs0 = np.zeros((K0, NP), np.float32)
        rhs0[0:E, :N] = es.T
        rhs0[E:E + D, :N] = xs.T
        rhs0[E + D, :] = 1.0
        m = dict(wmap)
        m["rhs0"] = rhs0
        in_maps.append(m)
    return in_maps


def kernel(x, embeddings, W0, b0, W1, b1, W2, b2, Wo, bo):
    x = np.asarray(x, np.float32)
    embeddings = np.asarray(embeddings, np.float32)
    wmap = _prep_weights(*[np.asarray(a, np.float32)
                           for a in (W0, b0, W1, b1, W2, b2, Wo, bo)])
    runner = _get_runner()
    outs = runner(_make_in_maps(x, embeddings, wmap))

    residuals = np.empty((B, LEN, D), np.float32)
    logdet = np.empty((B, LEN), np.float32)
    for k in range(NCORES):
        residuals[k * BC:(k + 1) * BC] = outs["res16"][k][:, :N].T.reshape(BC, LEN, D)
        logdet[k * BC:(k + 1) * BC] = outs["ldet"][k][0, :N].reshape(BC, LEN)
    return residuals, logdet
